# revision 2
# baseline (speedup 1.0000x reference)
"""Trainium2 Bass kernel for a 2-layer GCN (FCGraphGNN) over 8 NeuronCores.

Math (matches reference):
  ew' = [edge_attr; ones(N)]  (self loops), deg = segsum(ew', dst), dinv = deg^-1/2
  h1 = relu(segsum(dinv[src]*ew*dinv[dst] * (x@W1)[src]) + b1)
  h2 = relu(segsum(norm * (h1@W2)[src]) + b2)
  out = mean-pool-by-graph(h2) @ Wo + bo

Strategy:
  - Shard edges by dst across 8 cores (sorted by dst host-side).
  - Virtual node space: nodes packed into windows of <=64 consecutive nodes,
    each window has a fixed [T_SIDE lo-tiles | T_SIDE hi-tiles] slot layout so
    one SPMD program serves all cores (different data, same shapes).
  - Messages gathered with dma_gather (256B rows) from a per-core DRAM table
    H row-scaled by dinv[src]; dst-side dinv folded into the window epilogue.
  - Per 128-edge tile: DVE builds S = (iota==dst_slot)*ew; PE accumulates
    S.T @ M into PSUM per window (the segment-sum).
  - dinv + H1 all-gathered across cores; pooled partial sums all-reduced.
"""

import os
import sys

import numpy as np

sys.path.insert(0, "/opt/trn_rl_repo")

# ---------------------------------------------------------------- constants
N_NODES = 50000
N_EDGES = 3200000
N_GRAPHS = 50
IN_F = 5
HID = 64
OUT_F = 2
N_CORES = 8

SLOTS = 64          # dst nodes per window
T_SIDE = 17         # 128-edge tiles per (window, src-half)
CAP = T_SIDE * 128  # edge slots per (window, side)
GROUP_W = 4         # windows fetched per dma_gather pair
SG = 52             # graph one-hot width (50 graphs + 2 junk bins)


def _pack_host(x, edge_index, edge_attr, batch):
    """Pure index/layout preprocessing (numpy). Returns per-core input dicts
    plus the static plan (NW, DEG_K...)."""
    src = np.asarray(edge_index[0], dtype=np.int64)
    dst = np.asarray(edge_index[1], dtype=np.int64)
    ew = np.asarray(edge_attr, dtype=np.float32).reshape(-1)
    loop = np.arange(N_NODES, dtype=np.int64)
    src = np.concatenate([src, loop]).astype(np.int32)
    dst = np.concatenate([dst, loop]).astype(np.int32)
    ew = np.concatenate([ew, np.ones(N_NODES, np.float32)])
    E = src.shape[0]

    deg_cnt = np.bincount(dst, minlength=N_NODES).astype(np.int64)
    node_ptr = np.zeros(N_NODES + 1, np.int64)
    np.cumsum(deg_cnt, out=node_ptr[1:])
    order = np.argsort(dst, kind="stable")

    # core node boundaries balancing edge counts
    cum = node_ptr[1:]
    nb = [0]
    for c in range(1, N_CORES):
        nb.append(int(np.searchsorted(cum, c * E / N_CORES)))
    nb.append(N_NODES)
    nb = np.array(nb, np.int64)
    split_node = int(nb[4])  # src < split_node -> "lo" half of virtual space

    side_lo = src < split_node
    deg_lo = np.bincount(dst[side_lo], minlength=N_NODES).astype(np.int64)
    deg_hi = deg_cnt - deg_lo

    # window packing per core
    core_windows = []
    for c in range(N_CORES):
        wlist = []
        v = int(nb[c])
        end = int(nb[c + 1])
        while v < end:
            ws = v
            lo = hi = cnt = 0
            while (
                v < end
                and cnt < SLOTS
                and lo + deg_lo[v] <= CAP
                and hi + deg_hi[v] <= CAP
            ):
                lo += int(deg_lo[v])
                hi += int(deg_hi[v])
                cnt += 1
                v += 1
            wlist.append((ws, v))
        core_windows.append(wlist)

    NW = max(len(w) for w in core_windows)
    NW = (NW + 7) // 8 * 8  # multiple of GROUP_W and the h0-write batch
    assert NW <= 128, f"NW={NW} exceeds int16 index budget"
    NVC = NW * SLOTS
    NV = N_CORES * NVC
    NVH = NV // 2

    # vid map (node -> virtual id)
    node_vid = np.zeros(N_NODES, np.int32)
    for c in range(N_CORES):
        for w, (ws, we) in enumerate(core_windows[c]):
            node_vid[ws:we] = c * NVC + w * SLOTS + np.arange(we - ws, dtype=np.int32)

    DEG_K = int(deg_cnt.max())
    DEG_K = (DEG_K + 3) // 4 * 4

    # per-side dst-sorted edge lists + ptrs
    lo_edges = order[side_lo[order]]
    hi_edges = order[~side_lo[order]]
    lo_ptr = np.zeros(N_NODES + 1, np.int64)
    np.cumsum(deg_lo, out=lo_ptr[1:])
    hi_ptr = np.zeros(N_NODES + 1, np.int64)
    np.cumsum(deg_hi, out=hi_ptr[1:])

    NTILES = NW * 2 * T_SIDE
    NG = NW // GROUP_W
    GI = GROUP_W * CAP          # idxs per gather call
    IDXC = GI // 16

    vid_src = node_vid[src]

    # global position of each edge within its dst node's sorted run
    col_within = np.empty(E, np.int64)
    ar = np.arange(E, dtype=np.int64)
    col_within[order] = ar - node_ptr[dst[order]]

    # xt in virtual layout (shared by all cores)
    xt_virt = np.zeros((IN_F, NV), np.float32)
    xt_virt[:, node_vid] = np.asarray(x, np.float32).T

    batch_i = np.asarray(batch, np.int64)

    per_core = []
    for c in range(N_CORES):
        wlist = core_windows[c]
        ewp = np.zeros((NTILES, 128), np.float32)
        dstp = np.zeros((NTILES, 128), np.float32)
        idx_lo = np.zeros((NW, CAP), np.int16)
        idx_hi = np.zeros((NW, CAP), np.int16)
        gid = np.full((SLOTS, NW), 50.0, np.float32)
        ewdeg = np.zeros((NVC, DEG_K), np.float32)

        for w, (ws, we) in enumerate(wlist):
            for s, (edges, ptr, idxbuf, voff) in enumerate(
                ((lo_edges, lo_ptr, idx_lo, 0), (hi_edges, hi_ptr, idx_hi, NVH))
            ):
                ids = edges[ptr[ws] : ptr[we]]
                n = ids.shape[0]
                t0 = (w * 2 + s) * T_SIDE
                flat_ew = ewp.reshape(-1)
                flat_dst = dstp.reshape(-1)
                base = t0 * 128
                flat_ew[base : base + n] = ew[ids]
                flat_dst[base : base + n] = (dst[ids] - ws).astype(np.float32)
                idxbuf[w, :n] = (vid_src[ids] - voff).astype(np.int16)
            gid[: we - ws, w] = batch_i[ws:we].astype(np.float32)

        # padded per-node edge weights for the degree pass
        e_lo = int(node_ptr[nb[c]])
        e_hi = int(node_ptr[nb[c + 1]])
        es = order[e_lo:e_hi]
        rows = node_vid[dst[es]] - c * NVC
        ewdeg[rows, col_within[es]] = ew[es]
        rowdeg = np.zeros(NVC, np.int64)
        nr = node_vid[nb[c] : nb[c + 1]] - c * NVC
        rowdeg[nr] = deg_cnt[nb[c] : nb[c + 1]]
        ewdeg[rowdeg == 0, 0] = 1.0

        # wrap gather indices: [NG, 128, IDXC] (16-partition wrap, replicated)
        def wrap(a):
            g = a.reshape(NG, GI // 16, 16).transpose(0, 2, 1)  # [NG,16,IDXC]
            return np.ascontiguousarray(np.tile(g, (1, 8, 1)))

        per_core.append(
            dict(
                ewcols=np.ascontiguousarray(ewp.T),
                dstcols=np.ascontiguousarray(dstp.T),
                idxlo=wrap(idx_lo.reshape(-1)),
                idxhi=wrap(idx_hi.reshape(-1)),
                gid=np.ascontiguousarray(gid),
                ewdeg=ewdeg,
            )
        )

    plan = dict(
        NW=NW, NVC=NVC, NV=NV, NVH=NVH, DEG_K=DEG_K,
        NTILES=NTILES, NG=NG, GI=GI, IDXC=IDXC,
    )
    return per_core, plan, xt_virt


def _build_program(plan):
    import concourse.bacc as bacc
    import concourse.bass as bass
    import concourse.tile as tile
    from concourse import mybir
    from concourse.tile_rust import add_dep_helper

    f32 = mybir.dt.float32
    i16 = mybir.dt.int16
    Alu = mybir.AluOpType
    Act = mybir.ActivationFunctionType

    NW = plan["NW"]; NVC = plan["NVC"]; NV = plan["NV"]; NVH = plan["NVH"]
    DEG_K = plan["DEG_K"]; NTILES = plan["NTILES"]; NG = plan["NG"]
    GI = plan["GI"]; IDXC = plan["IDXC"]

    STAGE = int(os.environ.get("KSTAGE", "9"))
    NQ = int(os.environ.get("KNQ", "4"))
    nc = bacc.Bacc("TRN2", target_bir_lowering=False, debug=False,
                   num_devices=N_CORES, num_swdge_queues=NQ)

    xt = nc.declare_dram_parameter("xt", [IN_F, NV], f32, isOutput=False)
    w1 = nc.declare_dram_parameter("w1", [IN_F, HID], f32, isOutput=False)
    w2 = nc.declare_dram_parameter("w2", [HID, HID], f32, isOutput=False)
    wo = nc.declare_dram_parameter("wo", [HID, OUT_F], f32, isOutput=False)
    b1 = nc.declare_dram_parameter("b1", [SLOTS, HID], f32, isOutput=False)
    b2 = nc.declare_dram_parameter("b2", [SLOTS, HID], f32, isOutput=False)
    bo = nc.declare_dram_parameter("bo", [N_GRAPHS, OUT_F], f32, isOutput=False)
    ewdeg = nc.declare_dram_parameter("ewdeg", [NVC, DEG_K], f32, isOutput=False)
    ewcols = nc.declare_dram_parameter("ewcols", [128, NTILES], f32, isOutput=False)
    dstcols = nc.declare_dram_parameter("dstcols", [128, NTILES], f32, isOutput=False)
    idxlo = nc.declare_dram_parameter("idxlo", [NG, 128, IDXC], i16, isOutput=False)
    idxhi = nc.declare_dram_parameter("idxhi", [NG, 128, IDXC], i16, isOutput=False)
    gidp = nc.declare_dram_parameter("gid", [SLOTS, NW], f32, isOutput=False)
    out = nc.declare_dram_parameter("out", [N_GRAPHS, OUT_F], f32, isOutput=True)
    chain_in = nc.declare_dram_parameter("chain", [1, 4], f32, isOutput=False)
    chain_out = nc.declare_dram_parameter("chain_out", [1, 4], f32, isOutput=True)
    KDBG = int(os.environ.get("KDBG", "0"))
    if KDBG:
        dbg_dinv = nc.declare_dram_parameter("dbg_dinv", [SLOTS, NW], f32, isOutput=True)
        dbg_h0 = nc.declare_dram_parameter("dbg_h0", [NV, HID], f32, isOutput=True)
        dbg_h1 = nc.declare_dram_parameter("dbg_h1", [2048, HID], f32, isOutput=True)
        dbg_pool = nc.declare_dram_parameter("dbg_pool", [HID + 1, SG], f32, isOutput=True)
        dbg_mlo = nc.declare_dram_parameter("dbg_mlo", [128, 68, HID], f32, isOutput=True)

    groups = [list(range(N_CORES))]

    with tile.TileContext(nc) as tc:
        with (
            tc.tile_pool(name="dram", bufs=1, space="DRAM") as dram,
            tc.tile_pool(name="const", bufs=1) as cpool,
            tc.tile_pool(name="persist", bufs=1) as ppool,
        ):
            h0 = dram.tile([NV, HID], f32, tag="h0")
            h1loc = dram.tile([NVC, HID], f32, tag="h1loc")
            h1glob = dram.tile([NV, HID], f32, tag="h1glob")
            dinv_loc_d = dram.tile([SLOTS, NW], f32, tag="dinvloc")
            dinv_glob_d = dram.tile([N_CORES, SLOTS, NW], f32, tag="dinvglob")
            pool_in_d = dram.tile([HID + 1, SG], f32, tag="poolin")
            pool_out_d = dram.tile([HID + 1, SG], f32, tag="poolout")

            # ---- constants
            iota64 = cpool.tile([128, SLOTS], f32, tag="iota64")
            nc.gpsimd.iota(iota64[:], pattern=[[1, SLOTS]], base=0,
                           channel_multiplier=0,
                           allow_small_or_imprecise_dtypes=True)
            iota52 = cpool.tile([SLOTS, SG], f32, tag="iota52")
            nc.gpsimd.iota(iota52[:], pattern=[[1, SG]], base=0,
                           channel_multiplier=0,
                           allow_small_or_imprecise_dtypes=True)
            w1s = cpool.tile([IN_F, HID], f32, tag="w1s")
            nc.sync.dma_start(w1s[:], w1[:])
            w2s = cpool.tile([HID, HID], f32, tag="w2s")
            nc.sync.dma_start(w2s[:], w2[:])
            wos = cpool.tile([HID, OUT_F], f32, tag="wos")
            nc.sync.dma_start(wos[:], wo[:])
            b1s = cpool.tile([SLOTS, HID], f32, tag="b1s")
            nc.sync.dma_start(b1s[:], b1[:])
            b2s = cpool.tile([SLOTS, HID], f32, tag="b2s")
            nc.sync.dma_start(b2s[:], b2[:])
            bos = cpool.tile([N_GRAPHS, OUT_F], f32, tag="bos")
            nc.sync.dma_start(bos[:], bo[:])
            gids = cpool.tile([SLOTS, NW], f32, tag="gids")
            nc.sync.dma_start(gids[:], gidp[:])
            ewc = cpool.tile([128, NTILES], f32, tag="ewc")
            nc.sync.dma_start(ewc[:], ewcols[:])
            dstc = cpool.tile([128, NTILES], f32, tag="dstc")
            nc.sync.dma_start(dstc[:], dstcols[:])

            dinvw = ppool.tile([SLOTS, NW], f32, tag="dinvw")
            dinvg = ppool.tile([SLOTS, N_CORES, NW], f32, tag="dinvg")

            KAMP = int(os.environ.get("KAMP", "1"))
            KCC = int(os.environ.get("KCC", "1"))
            KNG = int(os.environ.get("KNG", "9999"))
            KGATHER = int(os.environ.get("KGATHER", "1"))

            # ---- message-passing layer (one pass over the edge tiles)
            def layer(l, rep, src_table, fence):
                lo_view = src_table[0:NVH, :]
                hi_view = src_table[NVH:NV, :]
                with (
                    tc.tile_pool(name=f"idx{l}_{rep}", bufs=4) as ipool,
                    tc.tile_pool(name=f"mbuf{l}_{rep}", bufs=2) as mpool,
                    tc.tile_pool(name=f"sbld{l}_{rep}", bufs=6) as spool,
                    tc.tile_pool(name=f"wpsum{l}_{rep}", bufs=4, space="PSUM") as wpool,
                    tc.tile_pool(name=f"epi{l}_{rep}", bufs=3) as epool,
                    tc.tile_pool(name=f"p2_{l}_{rep}", bufs=2, space="PSUM") as p2pool,
                    tc.tile_pool(name=f"gpool{l}_{rep}", bufs=1, space="PSUM") as gpool,
                ):
                    if l == 2:
                        pool_ps = gpool.tile([HID + 1, SG], f32, tag="poolps")
                    for g in range(min(NG, KNG)):
                        ilo = ipool.tile([128, IDXC], i16, tag="ilo")
                        nc.sync.dma_start(ilo[:], idxlo[g])
                        ihi = ipool.tile([128, IDXC], i16, tag="ihi")
                        nc.sync.dma_start(ihi[:], idxhi[g])
                        mlo = mpool.tile([128, GROUP_W * T_SIDE, HID], f32,
                                         tag="mlo")
                        mhi = mpool.tile([128, GROUP_W * T_SIDE, HID], f32,
                                         tag="mhi")
                        if KGATHER:
                            glo = nc.gpsimd.dma_gather(
                                mlo[:], lo_view, ilo[:], GI, GI, HID,
                                single_packet=False,
                                queue_num=(2 * g) % NQ,
                            )
                            ghi = nc.gpsimd.dma_gather(
                                mhi[:], hi_view, ihi[:], GI, GI, HID,
                                single_packet=False,
                                queue_num=(2 * g + 1) % NQ,
                            )
                            if fence is not None:
                                add_dep_helper(glo.ins, fence.ins,
                                               reason="gather src table ready")
                                add_dep_helper(ghi.ins, fence.ins,
                                               reason="gather src table ready")
                        else:
                            nc.sync.dma_start(
                                mlo[:],
                                src_table[0:GI, :].rearrange(
                                    "(b p) h -> p b h", p=128
                                ),
                            )
                            nc.sync.dma_start(
                                mhi[:],
                                src_table[0:GI, :].rearrange(
                                    "(b p) h -> p b h", p=128
                                ),
                            )
                        if KDBG and l == 1 and g == 0 and rep == 0:
                            nc.sync.dma_start(dbg_mlo[:], mlo[:])
                        for wl in range(GROUP_W):
                            w = g * GROUP_W + wl
                            ps = wpool.tile([SLOTS, HID], f32, tag="wps")
                            k = 0
                            for s, mb in ((0, mlo), (1, mhi)):
                                for ti in range(T_SIDE):
                                    t = (w * 2 + s) * T_SIDE + ti
                                    blk = wl * T_SIDE + ti
                                    S = spool.tile([128, SLOTS], f32, tag="S")
                                    nc.vector.tensor_scalar(
                                        out=S[:], in0=iota64[:],
                                        scalar1=dstc[:, t : t + 1],
                                        scalar2=ewc[:, t : t + 1],
                                        op0=Alu.is_equal, op1=Alu.mult,
                                    )
                                    first = k == 0
                                    last = k == 2 * T_SIDE - 1
                                    if l == 1:
                                        nc.tensor.matmul(
                                            out=ps[:], lhsT=S[:],
                                            rhs=mb[:, blk, :],
                                            start=first, stop=last,
                                        )
                                    else:
                                        nc.tensor.matmul(
                                            out=ps[:], lhsT=mb[:, blk, :],
                                            rhs=S[:],
                                            start=first, stop=last,
                                        )
                                    k += 1
                            dv = dinvw[:, w : w + 1]
                            if l == 1:
                                u = epool.tile([SLOTS, HID], f32, tag="u1")
                                nc.vector.tensor_scalar(
                                    out=u[:], in0=ps[:], scalar1=dv,
                                    scalar2=None, op0=Alu.mult,
                                )
                                nc.vector.tensor_tensor(
                                    out=u[:], in0=u[:], in1=b1s[:], op=Alu.add,
                                )
                                nc.vector.tensor_scalar(
                                    out=u[:], in0=u[:], scalar1=0.0,
                                    scalar2=None, op0=Alu.max,
                                )
                                uh = epool.tile([SLOTS, HID], f32, tag="uh")
                                nc.vector.tensor_scalar(
                                    out=uh[:], in0=u[:], scalar1=dv,
                                    scalar2=None, op0=Alu.mult,
                                )
                                nc.sync.dma_start(
                                    h1loc[w * SLOTS : (w + 1) * SLOTS, :],
                                    uh[:],
                                )
                            else:
                                aggT = epool.tile([HID, SLOTS], f32, tag="aggT")
                                nc.vector.tensor_copy(aggT[:], ps[:])
                                ps2 = p2pool.tile([SLOTS, HID], f32, tag="ps2")
                                nc.tensor.matmul(
                                    out=ps2[:], lhsT=aggT[:], rhs=w2s[:],
                                    start=True, stop=True,
                                )
                                u = epool.tile([SLOTS, HID + 1], f32, tag="u2")
                                nc.vector.memset(u[:, HID : HID + 1], 1.0)
                                nc.vector.tensor_scalar(
                                    out=u[:, 0:HID], in0=ps2[:], scalar1=dv,
                                    scalar2=None, op0=Alu.mult,
                                )
                                nc.vector.tensor_tensor(
                                    out=u[:, 0:HID], in0=u[:, 0:HID],
                                    in1=b2s[:], op=Alu.add,
                                )
                                nc.vector.tensor_scalar(
                                    out=u[:, 0:HID], in0=u[:, 0:HID],
                                    scalar1=0.0, scalar2=None, op0=Alu.max,
                                )
                                Sg = epool.tile([SLOTS, SG], f32, tag="Sg")
                                nc.vector.tensor_scalar(
                                    out=Sg[:], in0=iota52[:],
                                    scalar1=gids[:, w : w + 1],
                                    scalar2=None, op0=Alu.is_equal,
                                )
                                nc.tensor.matmul(
                                    out=pool_ps[:], lhsT=u[:], rhs=Sg[:],
                                    start=(w == 0),
                                    stop=(w == min(NG, KNG) * GROUP_W - 1),
                                )
                    if l == 2 and KNG >= NG:
                        pst = epool.tile([HID + 1, SG], f32, tag="pst")
                        nc.vector.tensor_copy(pst[:], pool_ps[:])
                        nc.sync.dma_start(pool_in_d[:], pst[:])

            # ---- one full pipeline iteration (repeated KAMP x for timing)
            for rep in range(KAMP):
                # degree pass -> local dinv
                with tc.tile_pool(name=f"deg{rep}", bufs=1) as dpool:
                    degt = dpool.tile([SLOTS, NW, DEG_K], f32, tag="degt")
                    nc.sync.dma_start(
                        degt[:], ewdeg[:].rearrange("(w s) k -> s w k", s=SLOTS)
                    )
                    deg = dpool.tile([SLOTS, NW], f32, tag="deg")
                    nc.vector.tensor_reduce(
                        out=deg[:], in_=degt[:], axis=mybir.AxisListType.X,
                        op=Alu.add,
                    )
                    rec = dpool.tile([SLOTS, NW], f32, tag="rec")
                    nc.vector.reciprocal(rec[:], deg[:])
                    nc.scalar.activation(dinvw[:], rec[:], Act.Sqrt)
                    nc.sync.dma_start(dinv_loc_d[:], dinvw[:])

                # all-gather dinv
                if KCC:
                    nc.gpsimd.collective_compute(
                        "AllGather", Alu.bypass, replica_groups=groups,
                        ins=[dinv_loc_d[:].rearrange("s w -> (s w)")],
                        outs=[dinv_glob_d[:].rearrange("r s w -> (r s w)")],
                    )
                else:
                    for _r in range(N_CORES):
                        nc.sync.dma_start(dinv_glob_d[_r], dinv_loc_d[:])
                nc.sync.dma_start(
                    dinvg[:], dinv_glob_d[:].rearrange("r s w -> s r w")
                )

                # preamble: h0 = dinv * (x @ W1), all rows, per core
                if STAGE >= 1:
                    with (
                        tc.tile_pool(name=f"pre{rep}", bufs=2) as prepool,
                        tc.tile_pool(name=f"prepsum{rep}", bufs=4,
                                     space="PSUM") as pspool,
                        tc.tile_pool(name=f"prestage{rep}", bufs=2) as stpool,
                    ):
                        BW = 8  # window tiles per h0 write
                        h0_writes = []
                        for r in range(N_CORES):
                            xts = prepool.tile([IN_F, NVC], f32, tag="xts")
                            nc.sync.dma_start(
                                xts[:], xt[:, r * NVC : (r + 1) * NVC]
                            )
                            for wb in range(NW // BW):
                                stage = stpool.tile([SLOTS, BW, HID], f32,
                                                    tag="h0st")
                                for j in range(BW):
                                    w = wb * BW + j
                                    ps = pspool.tile([SLOTS, HID], f32,
                                                     tag="prepsum")
                                    nc.tensor.matmul(
                                        out=ps[:],
                                        lhsT=xts[:, w * SLOTS : (w + 1) * SLOTS],
                                        rhs=w1s[:],
                                        start=True, stop=True,
                                    )
                                    nc.scalar.activation(
                                        stage[:, j, :], ps[:], Act.Copy,
                                        scale=dinvg[:, r, w : w + 1],
                                    )
                                row0 = (r * NW + wb * BW) * SLOTS
                                h0_writes.append(
                                    nc.sync.dma_start(
                                        h0[row0 : row0 + BW * SLOTS, :].rearrange(
                                            "(b s) h -> s b h", s=SLOTS
                                        ),
                                        stage[:],
                                    )
                                )

                # fence: gathers reading h0 wait on all its writes
                if STAGE >= 1:
                    fence0 = nc.gpsimd.engine_nop()
                    for wi in h0_writes:
                        add_dep_helper(fence0.ins, wi.ins,
                                       reason="h0 table ready before gathers")
                else:
                    fence0 = None

                if STAGE >= 2:
                    layer(1, rep, h0, fence0)

                if STAGE >= 3:
                    cc_h1 = nc.gpsimd.collective_compute(
                        "AllGather", Alu.bypass, replica_groups=groups,
                        ins=[h1loc[:].rearrange("a b -> (a b)")],
                        outs=[h1glob[:].rearrange("a b -> (a b)")],
                    )

                if STAGE >= 4:
                    layer(2, rep, h1glob, cc_h1)

            if STAGE < 4:
                # keep the tail runnable: zero the pooled partials
                with tc.tile_pool(name="dummy", bufs=1) as dpool2:
                    osb0 = dpool2.tile([HID + 1, SG], f32, tag="osb0")
                    nc.vector.memset(osb0[:], 0.0)
                    nc.sync.dma_start(pool_in_d[:], osb0[:])

            if KDBG:
                nc.sync.dma_start(dbg_dinv[:], dinvw[:])
                nc.sync.dma_start(dbg_h0[:], h0[:])
                nc.sync.dma_start(dbg_h1[:], h1loc[0:2048, :])
                nc.sync.dma_start(dbg_pool[:], pool_in_d[:])

            # ---- pooled partial sums -> all-reduce -> final linear
            if KCC:
                nc.gpsimd.collective_compute(
                    "AllReduce", Alu.add, replica_groups=groups,
                    ins=[pool_in_d[:]], outs=[pool_out_d[:]],
                )
            else:
                nc.sync.dma_start(pool_out_d[:], pool_in_d[:])
            with (
                tc.tile_pool(name="fin", bufs=1) as fpool,
                tc.tile_pool(name="finps", bufs=1, space="PSUM") as fpsum,
            ):
                pr = fpool.tile([HID + 1, SG], f32, tag="pr")
                nc.sync.dma_start(pr[:], pool_out_d[:])
                cm = fpool.tile([1, SG], f32, tag="cm")
                nc.vector.tensor_scalar(
                    out=cm[:], in0=pr[HID : HID + 1, :], scalar1=1.0,
                    scalar2=None, op0=Alu.max,
                )
                rcp = fpool.tile([1, SG], f32, tag="rcp")
                nc.vector.reciprocal(rcp[:], cm[:])
                rcpb = fpool.tile([HID, SG], f32, tag="rcpb")
                nc.gpsimd.partition_broadcast(rcpb[:], rcp[:])
                pooledT = fpool.tile([HID, N_GRAPHS], f32, tag="pooledT")
                nc.vector.tensor_tensor(
                    out=pooledT[:], in0=pr[0:HID, 0:N_GRAPHS],
                    in1=rcpb[0:HID, 0:N_GRAPHS],
                    op=Alu.mult,
                )
                pso = fpsum.tile([N_GRAPHS, OUT_F], f32, tag="pso")
                nc.tensor.matmul(
                    out=pso[:], lhsT=pooledT[:], rhs=wos[:],
                    start=True, stop=True,
                )
                osb = fpool.tile([N_GRAPHS, OUT_F], f32, tag="osb")
                nc.vector.tensor_tensor(
                    out=osb[:], in0=pso[:],
                    in1=bos[:],
                    op=Alu.add,
                )
                nc.sync.dma_start(out[:], osb[:])
                chs = fpool.tile([1, 4], f32, tag="chs")
                nc.sync.dma_start(chs[:], chain_in[:])
                nc.vector.tensor_scalar_add(chs[:], chs[:], 1.0)
                nc.sync.dma_start(chain_out[:], chs[:])

    nc.compile()
    return nc


def _make_runner(nc, repeat=1):
    """Cached-jit SPMD runner modeled on bass2jax.run_bass_via_pjrt, for
    benchmarking: returns (fn, prep) where prep(in_maps) device-puts inputs
    once and fn(args) executes the compiled NEFF on all 8 cores."""
    import jax
    import numpy as np
    from jax.experimental.shard_map import shard_map
    from jax.sharding import Mesh, NamedSharding, PartitionSpec

    from concourse import bass2jax, mybir
    from concourse.bass2jax import (
        _bass_exec_p, install_neuronx_cc_hook, partition_id_tensor,
    )

    install_neuronx_cc_hook()
    pname = nc.partition_id_tensor.name if nc.partition_id_tensor else None
    in_names, out_names, out_avals, zero_outs = [], [], [], []
    for alloc in nc.m.functions[0].allocations:
        if not isinstance(alloc, mybir.MemoryLocationSet):
            continue
        name = alloc.memorylocations[0].name
        if alloc.kind == "ExternalInput":
            if name == pname:
                continue
            in_names.append(name)
        elif alloc.kind == "ExternalOutput":
            shape = tuple(alloc.tensor_shape)
            dtype = mybir.dt.np(alloc.dtype)
            out_names.append(name)
            out_avals.append(jax.core.ShapedArray(shape, dtype))
            zero_outs.append(np.zeros(shape, dtype))
    n_params = len(in_names)
    all_names = in_names + out_names
    if pname is not None:
        all_names = all_names + [pname]

    chain_i = in_names.index("chain") if "chain" in in_names else None
    chain_o = out_names.index("chain_out") if "chain_out" in out_names else None

    def _body(*args):
        operands = list(args)
        if pname is not None:
            operands.append(partition_id_tensor())
        for _ in range(repeat):
            outs = _bass_exec_p.bind(
                *operands,
                out_avals=tuple(out_avals),
                in_names=tuple(all_names),
                out_names=tuple(out_names),
                lowering_input_output_aliases=(),
                sim_require_finite=True,
                sim_require_nnan=True,
                nc=nc,
            )
            if chain_i is not None:
                operands[chain_i] = outs[chain_o]
        return tuple(outs)

    devices = jax.devices()[:N_CORES]
    mesh = Mesh(np.asarray(devices), ("core",))
    spec = PartitionSpec("core")
    n_all = n_params + len(out_names)
    fn = jax.jit(
        shard_map(
            _body, mesh=mesh, in_specs=(spec,) * n_all,
            out_specs=(spec,) * len(out_names), check_rep=False,
        ),
        keep_unused=True,
    )

    def prep(in_maps):
        sharding = NamedSharding(mesh, spec)
        args = []
        for i, name in enumerate(in_names):
            cat = np.concatenate([np.asarray(m[name]) for m in in_maps], axis=0)
            args.append(jax.device_put(cat, sharding))
        for z in zero_outs:
            cat = np.zeros((N_CORES * z.shape[0], *z.shape[1:]), z.dtype)
            args.append(jax.device_put(cat, sharding))
        return args

    def unpack(outs):
        return {
            name: np.asarray(outs[i]).reshape(N_CORES, *out_avals[i].shape)[0]
            for i, name in enumerate(out_names)
        }

    return fn, prep, unpack


def kernel(x, edge_index, edge_attr, batch, W1, b1, W2, b2, Wo, bo, **_):
    per_core, plan, xt_virt = _pack_host(x, edge_index, edge_attr, batch)
    nc = _build_program(plan)

    common = dict(
        chain=np.zeros((1, 4), np.float32),
        xt=xt_virt,
        w1=np.asarray(W1, np.float32),
        w2=np.asarray(W2, np.float32),
        wo=np.asarray(Wo, np.float32),
        b1=np.tile(np.asarray(b1, np.float32).reshape(1, -1), (SLOTS, 1)),
        b2=np.tile(np.asarray(b2, np.float32).reshape(1, -1), (SLOTS, 1)),
        bo=np.tile(np.asarray(bo, np.float32).reshape(1, -1), (N_GRAPHS, 1)),
    )
    in_maps = []
    for c in range(N_CORES):
        m = dict(common)
        m.update(per_core[c])
        in_maps.append(m)

    from concourse.bass_utils import run_bass_kernel_spmd

    res = run_bass_kernel_spmd(nc, in_maps, list(range(N_CORES)))
    out = res.results[0]["out"]
    kernel.last_exec_time_ns = res.exec_time_ns
    kernel.last_results = res.results
    kernel.last_res = res
    return np.asarray(out, np.float32)


kernel.last_exec_time_ns = None



# revision 3
# speedup vs baseline: 1.2415x; 1.2415x over previous
"""Trainium2 Bass kernel v3 for the 2-layer GCN (FCGraphGNN) over 8 NeuronCores.

v1 architecture (edge-sharded by dst, 64-node windows, dma_gather of 256B
message rows from per-core DRAM tables, one-hot S scatter matmuls, PSUM
segment-sum), with the measured bottlenecks fixed:
  - S one-hot tiles are fully host-precomputed (f32) and streamed from DRAM
    instead of 7k per-tile DVE builds (~4ms of DVE instruction overhead).
  - Epilogues, preamble scaling, and pooling are batched 8 windows wide
    (3 wide DVE ops per 8 windows instead of ~5 per window).
  - Graph-mean-pool counts are host-precomputed index data.
GPSIMD descriptor generation for the gathers (~7.3ns/edge) remains the
critical path; everything else is hidden underneath it.
"""

import os
import sys

import numpy as np

sys.path.insert(0, "/opt/trn_rl_repo")

N_NODES = 50000
N_EDGES = 3200000
N_GRAPHS = 50
IN_F = 5
HID = 64
OUT_F = 2
N_CORES = 8

SLOTS = 64          # dst nodes per window
T_SIDE = 17         # 128-edge tiles per (window, src-half)
CAP = T_SIDE * 128  # edge slots per (window, side)
NTW = 2 * T_SIDE    # tiles per window
GROUP_W = 2         # windows fetched per dma_gather pair
WB = 8              # windows per epilogue batch
SG = 52             # graph one-hot width (50 graphs + 2 junk bins)


def _pack_host(x, edge_index, edge_attr, batch):
    src = np.asarray(edge_index[0], dtype=np.int64)
    dst = np.asarray(edge_index[1], dtype=np.int64)
    ew = np.asarray(edge_attr, dtype=np.float32).reshape(-1)
    loop = np.arange(N_NODES, dtype=np.int64)
    src = np.concatenate([src, loop]).astype(np.int32)
    dst = np.concatenate([dst, loop]).astype(np.int32)
    ew = np.concatenate([ew, np.ones(N_NODES, np.float32)])
    E = src.shape[0]

    deg_cnt = np.bincount(dst, minlength=N_NODES).astype(np.int64)
    node_ptr = np.zeros(N_NODES + 1, np.int64)
    np.cumsum(deg_cnt, out=node_ptr[1:])
    order = np.argsort(dst, kind="stable")

    # core node boundaries balancing edge counts
    cum = node_ptr[1:]
    nb = [0]
    for c in range(1, N_CORES):
        nb.append(int(np.searchsorted(cum, c * E / N_CORES)))
    nb.append(N_NODES)
    nb = np.array(nb, np.int64)
    split_node = int(nb[4])  # src < split_node -> "lo" half of virtual space

    side_lo = src < split_node
    deg_lo = np.bincount(dst[side_lo], minlength=N_NODES).astype(np.int64)
    deg_hi = deg_cnt - deg_lo

    # window packing per core
    core_windows = []
    for c in range(N_CORES):
        wlist = []
        v = int(nb[c])
        end = int(nb[c + 1])
        while v < end:
            ws = v
            lo = hi = cnt = 0
            while (
                v < end
                and cnt < SLOTS
                and lo + deg_lo[v] <= CAP
                and hi + deg_hi[v] <= CAP
            ):
                lo += int(deg_lo[v])
                hi += int(deg_hi[v])
                cnt += 1
                v += 1
            wlist.append((ws, v))
        core_windows.append(wlist)

    NW = max(len(w) for w in core_windows)
    NW = (NW + WB - 1) // WB * WB
    assert NW <= 128, f"NW={NW} exceeds int16 index budget"
    NVC = NW * SLOTS
    NV = N_CORES * NVC
    NVH = NV // 2

    # vid map (node -> virtual id)
    node_vid = np.zeros(N_NODES, np.int32)
    for c in range(N_CORES):
        for w, (ws, we) in enumerate(core_windows[c]):
            node_vid[ws:we] = c * NVC + w * SLOTS + np.arange(we - ws,
                                                             dtype=np.int32)

    DEG_K = int(deg_cnt.max())
    DEG_K = (DEG_K + 3) // 4 * 4

    # per-side dst-sorted edge lists + ptrs
    lo_edges = order[side_lo[order]]
    hi_edges = order[~side_lo[order]]
    lo_ptr = np.zeros(N_NODES + 1, np.int64)
    np.cumsum(deg_lo, out=lo_ptr[1:])
    hi_ptr = np.zeros(N_NODES + 1, np.int64)
    np.cumsum(deg_hi, out=hi_ptr[1:])

    NTILES = NW * NTW
    NG = NW // GROUP_W
    GI = GROUP_W * CAP          # idxs per gather call
    IDXC = GI // 16

    vid_src = node_vid[src]

    # global position of each edge within its dst node's sorted run
    col_within = np.empty(E, np.int64)
    ar = np.arange(E, dtype=np.int64)
    col_within[order] = ar - node_ptr[dst[order]]

    # xt in virtual layout (shared by all cores)
    xt_virt = np.zeros((IN_F, NV), np.float32)
    xt_virt[:, node_vid] = np.asarray(x, np.float32).T

    batch_i = np.asarray(batch, np.int64)

    per_core = []
    for c in range(N_CORES):
        wlist = core_windows[c]
        sfull = np.zeros((128, NTILES * SLOTS), np.float32)
        idx_lo = np.zeros((NW, CAP), np.int16)
        idx_hi = np.zeros((NW, CAP), np.int16)
        gid = np.full((SLOTS, NW), 50.0, np.float32)
        ewdeg = np.zeros((NVC, DEG_K), np.float32)

        for w, (ws, we) in enumerate(wlist):
            for s, (edges, ptr, idxbuf, voff) in enumerate(
                ((lo_edges, lo_ptr, idx_lo, 0), (hi_edges, hi_ptr, idx_hi,
                                                 NVH))
            ):
                ids = edges[ptr[ws]:ptr[we]]
                n = ids.shape[0]
                t0 = (w * 2 + s) * T_SIDE
                j = np.arange(n)
                rows = j % 128
                cols = (t0 + j // 128) * SLOTS + (dst[ids] - ws)
                sfull[rows, cols] = ew[ids]
                idxbuf[w, :n] = (vid_src[ids] - voff).astype(np.int16)
            gid[: we - ws, w] = batch_i[ws:we].astype(np.float32)

        # padded per-node edge weights for the degree pass
        e_lo = int(node_ptr[nb[c]])
        e_hi = int(node_ptr[nb[c + 1]])
        es = order[e_lo:e_hi]
        rows = node_vid[dst[es]] - c * NVC
        ewdeg[rows, col_within[es]] = ew[es]
        rowdeg = np.zeros(NVC, np.int64)
        nr = node_vid[nb[c]:nb[c + 1]] - c * NVC
        rowdeg[nr] = deg_cnt[nb[c]:nb[c + 1]]
        ewdeg[rowdeg == 0, 0] = 1.0

        # wrap gather indices: [NG, 128, IDXC] (16-partition wrap, replicated)
        def wrap(a):
            g = a.reshape(NG, GI // 16, 16).transpose(0, 2, 1)  # [NG,16,IDXC]
            return np.ascontiguousarray(np.tile(g, (1, 8, 1)))

        per_core.append(
            dict(
                sfull=sfull,
                idxlo=wrap(idx_lo.reshape(-1)),
                idxhi=wrap(idx_hi.reshape(-1)),
                gid=np.ascontiguousarray(gid),
                ewdeg=ewdeg,
            )
        )

    cnt = np.bincount(batch_i, minlength=N_GRAPHS).astype(np.float32)
    rcnt = np.zeros(SG, np.float32)
    rcnt[:N_GRAPHS] = 1.0 / np.maximum(cnt, 1.0)

    plan = dict(
        NW=NW, NVC=NVC, NV=NV, NVH=NVH, DEG_K=DEG_K,
        NTILES=NTILES, NG=NG, GI=GI, IDXC=IDXC,
    )
    return per_core, plan, xt_virt, rcnt


def _build_program(plan):
    import concourse.bacc as bacc
    import concourse.tile as tile
    from concourse import mybir
    from concourse.tile_rust import add_dep_helper

    f32 = mybir.dt.float32
    i16 = mybir.dt.int16
    Alu = mybir.AluOpType
    Act = mybir.ActivationFunctionType

    NW = plan["NW"]; NVC = plan["NVC"]; NV = plan["NV"]; NVH = plan["NVH"]
    DEG_K = plan["DEG_K"]; NTILES = plan["NTILES"]; NG = plan["NG"]
    GI = plan["GI"]; IDXC = plan["IDXC"]

    NQ = int(os.environ.get("KNQ", "4"))
    nc = bacc.Bacc("TRN2", target_bir_lowering=False, debug=False,
                   num_devices=N_CORES, num_swdge_queues=NQ)

    xt = nc.declare_dram_parameter("xt", [IN_F, NV], f32, isOutput=False)
    w1 = nc.declare_dram_parameter("w1", [IN_F, HID], f32, isOutput=False)
    w2 = nc.declare_dram_parameter("w2", [HID, HID], f32, isOutput=False)
    wo = nc.declare_dram_parameter("wo", [HID, OUT_F], f32, isOutput=False)
    b1bc = nc.declare_dram_parameter("b1bc", [SLOTS, HID], f32, isOutput=False)
    b2bc = nc.declare_dram_parameter("b2bc", [SLOTS, HID], f32, isOutput=False)
    bo52 = nc.declare_dram_parameter("bo52", [SG, OUT_F], f32, isOutput=False)
    rcntp = nc.declare_dram_parameter("rcnt", [SLOTS, SG], f32, isOutput=False)
    ewdeg = nc.declare_dram_parameter("ewdeg", [NVC, DEG_K], f32,
                                      isOutput=False)
    sfullp = nc.declare_dram_parameter("sfull", [128, NTILES * SLOTS], f32,
                                       isOutput=False)
    idxlo = nc.declare_dram_parameter("idxlo", [NG, 128, IDXC], i16,
                                      isOutput=False)
    idxhi = nc.declare_dram_parameter("idxhi", [NG, 128, IDXC], i16,
                                      isOutput=False)
    gidp = nc.declare_dram_parameter("gid", [SLOTS, NW], f32, isOutput=False)
    out = nc.declare_dram_parameter("out", [N_GRAPHS, OUT_F], f32,
                                    isOutput=True)

    groups = [list(range(N_CORES))]

    with tile.TileContext(nc) as tc:
        with (
            tc.tile_pool(name="dram", bufs=1, space="DRAM") as dram,
            tc.tile_pool(name="const", bufs=1) as cpool,
            tc.tile_pool(name="persist", bufs=1) as ppool,
        ):
            h0 = dram.tile([NV, HID], f32, tag="h0")
            h1loc = dram.tile([NVC, HID], f32, tag="h1loc")
            h1glob = dram.tile([NV, HID], f32, tag="h1glob")
            dinv_loc_d = dram.tile([SLOTS, NW], f32, tag="dinvloc")
            dinv_glob_d = dram.tile([N_CORES, SLOTS, NW], f32, tag="dinvglob")
            pool_in_d = dram.tile([HID, SG], f32, tag="poolin")
            pool_out_d = dram.tile([HID, SG], f32, tag="poolout")

            # ---- constants
            iota52 = cpool.tile([SLOTS, SG], f32, tag="iota52")
            nc.gpsimd.iota(iota52[:], pattern=[[1, SG]], base=0,
                           channel_multiplier=0,
                           allow_small_or_imprecise_dtypes=True)
            w1s = cpool.tile([IN_F, HID], f32, tag="w1s")
            nc.sync.dma_start(w1s[:], w1[:])
            w2s = cpool.tile([HID, HID], f32, tag="w2s")
            nc.sync.dma_start(w2s[:], w2[:])
            wos = cpool.tile([HID, OUT_F], f32, tag="wos")
            nc.sync.dma_start(wos[:], wo[:])
            b1s = cpool.tile([SLOTS, HID], f32, tag="b1s")
            nc.sync.dma_start(b1s[:], b1bc[:])
            b2s = cpool.tile([SLOTS, HID], f32, tag="b2s")
            nc.sync.dma_start(b2s[:], b2bc[:])
            bos = cpool.tile([SG, OUT_F], f32, tag="bos")
            nc.sync.dma_start(bos[:], bo52[:])
            rcntb = cpool.tile([SLOTS, SG], f32, tag="rcntb")
            nc.sync.dma_start(rcntb[:], rcntp[:])
            gids = cpool.tile([SLOTS, NW], f32, tag="gids")
            nc.sync.dma_start(gids[:], gidp[:])

            dinvw = ppool.tile([SLOTS, NW], f32, tag="dinvw")
            dinvg = ppool.tile([SLOTS, N_CORES, NW], f32, tag="dinvg")

            # ---- degree pass -> local dinv
            with tc.tile_pool(name="deg", bufs=1) as dpool:
                degt = dpool.tile([SLOTS, NW, DEG_K], f32, tag="degt")
                nc.sync.dma_start(
                    degt[:], ewdeg[:].rearrange("(w s) k -> s w k", s=SLOTS)
                )
                deg = dpool.tile([SLOTS, NW], f32, tag="deg")
                nc.vector.tensor_reduce(
                    out=deg[:], in_=degt[:], axis=mybir.AxisListType.X,
                    op=Alu.add,
                )
                rec = dpool.tile([SLOTS, NW], f32, tag="rec")
                nc.vector.reciprocal(rec[:], deg[:])
                nc.scalar.activation(dinvw[:], rec[:], Act.Sqrt)
                nc.sync.dma_start(dinv_loc_d[:], dinvw[:])

            # all-gather dinv
            nc.gpsimd.collective_compute(
                "AllGather", Alu.bypass, replica_groups=groups,
                ins=[dinv_loc_d[:].rearrange("s w -> (s w)")],
                outs=[dinv_glob_d[:].rearrange("r s w -> (r s w)")],
            )
            nc.sync.dma_start(
                dinvg[:], dinv_glob_d[:].rearrange("r s w -> s r w")
            )

            # ---- preamble: h0 = dinv * (x @ W1), all rows, per core
            with (
                tc.tile_pool(name="pre", bufs=2) as prepool,
                tc.tile_pool(name="prepsum", bufs=2, space="PSUM") as pspool,
                tc.tile_pool(name="prestage", bufs=2) as stpool,
            ):
                BW = 8  # window tiles per h0 write
                h0_writes = []
                for r in range(N_CORES):
                    xts = prepool.tile([IN_F, NVC], f32, tag="xts")
                    nc.sync.dma_start(
                        xts[:], xt[:, r * NVC:(r + 1) * NVC]
                    )
                    for wb in range(NW // BW):
                        bank = pspool.tile([SLOTS, BW * HID], f32,
                                           tag="prebank")
                        for j in range(BW):
                            w = wb * BW + j
                            nc.tensor.matmul(
                                out=bank[:, j * HID:(j + 1) * HID],
                                lhsT=xts[:, w * SLOTS:(w + 1) * SLOTS],
                                rhs=w1s[:],
                                start=True, stop=True,
                            )
                        stage = stpool.tile([SLOTS, BW, HID], f32, tag="h0st")
                        dexp = (
                            dinvg[:, r, wb * BW:(wb + 1) * BW]
                            .unsqueeze(2).broadcast_to([SLOTS, BW, HID])
                        )
                        nc.vector.tensor_tensor(
                            out=stage[:],
                            in0=bank[:].rearrange("s (b h) -> s b h", h=HID),
                            in1=dexp, op=Alu.mult,
                        )
                        row0 = (r * NW + wb * BW) * SLOTS
                        h0_writes.append(
                            nc.sync.dma_start(
                                h0[row0:row0 + BW * SLOTS, :].rearrange(
                                    "(b s) h -> s b h", s=SLOTS
                                ),
                                stage[:],
                            )
                        )

            # fence: gathers reading h0 wait on all its writes
            fence0 = nc.gpsimd.engine_nop()
            for wi in h0_writes:
                add_dep_helper(fence0.ins, wi.ins,
                               reason="h0 table ready before gathers")

            # ---- message-passing layer (one pass over the edge tiles)
            def layer(l, src_table, fence):
                lo_view = src_table[0:NVH, :]
                hi_view = src_table[NVH:NV, :]
                with (
                    tc.tile_pool(name=f"idx{l}", bufs=4) as ipool,
                    tc.tile_pool(name=f"mbuf{l}", bufs=2) as mpool,
                    tc.tile_pool(name=f"sstr{l}", bufs=2) as spool,
                    tc.tile_pool(name=f"acc{l}", bufs=2, space="PSUM") as apool,
                    tc.tile_pool(name=f"acc2{l}", bufs=2, space="PSUM") as bpool,
                    tc.tile_pool(name=f"epi{l}", bufs=2) as epool,
                    tc.tile_pool(name=f"gp{l}", bufs=1, space="PSUM") as gpool,
                ):
                    if l == 2:
                        pool_ps = gpool.tile([HID, SG], f32, tag="poolps")
                    for wbi in range(NW // WB):
                        acc = apool.tile([SLOTS, WB * HID], f32, tag="accA")
                        for gg in range(WB // GROUP_W):
                            g = wbi * (WB // GROUP_W) + gg
                            ilo = ipool.tile([128, IDXC], i16, tag="ilo")
                            nc.sync.dma_start(ilo[:], idxlo[g])
                            ihi = ipool.tile([128, IDXC], i16, tag="ihi")
                            nc.sync.dma_start(ihi[:], idxhi[g])
                            mlo = mpool.tile(
                                [128, GROUP_W * T_SIDE, HID], f32, tag="mlo"
                            )
                            mhi = mpool.tile(
                                [128, GROUP_W * T_SIDE, HID], f32, tag="mhi"
                            )
                            glo = nc.gpsimd.dma_gather(
                                mlo[:], lo_view, ilo[:], GI, GI, HID,
                                single_packet=False,
                                queue_num=(2 * g) % NQ,
                            )
                            ghi = nc.gpsimd.dma_gather(
                                mhi[:], hi_view, ihi[:], GI, GI, HID,
                                single_packet=False,
                                queue_num=(2 * g + 1) % NQ,
                            )
                            if fence is not None:
                                add_dep_helper(glo.ins, fence.ins,
                                               reason="src table ready")
                                add_dep_helper(ghi.ins, fence.ins,
                                               reason="src table ready")
                            ssb = spool.tile(
                                [128, GROUP_W * NTW * SLOTS], f32, tag="ssb"
                            )
                            c0 = g * GROUP_W * NTW * SLOTS
                            nc.sync.dma_start(
                                ssb[:],
                                sfullp[:, c0:c0 + GROUP_W * NTW * SLOTS],
                            )
                            for wl in range(GROUP_W):
                                wj = gg * GROUP_W + wl  # window in batch
                                aslice = acc[:, wj * HID:(wj + 1) * HID]
                                k = 0
                                for s, mb in ((0, mlo), (1, mhi)):
                                    for ti in range(T_SIDE):
                                        blk = wl * T_SIDE + ti
                                        scol = (
                                            (wl * NTW + s * T_SIDE + ti)
                                            * SLOTS
                                        )
                                        first = k == 0
                                        last = k == NTW - 1
                                        if l == 1:
                                            nc.tensor.matmul(
                                                out=aslice,
                                                lhsT=ssb[:,
                                                         scol:scol + SLOTS],
                                                rhs=mb[:, blk, :],
                                                start=first, stop=last,
                                            )
                                        else:
                                            nc.tensor.matmul(
                                                out=aslice,
                                                lhsT=mb[:, blk, :],
                                                rhs=ssb[:,
                                                        scol:scol + SLOTS],
                                                start=first, stop=last,
                                            )
                                        k += 1
                        # ---- batched epilogue over WB windows
                        w0 = wbi * WB
                        dexp = (
                            dinvw[:, w0:w0 + WB]
                            .unsqueeze(2).broadcast_to([SLOTS, WB, HID])
                        )
                        accv = acc[:].rearrange("s (b h) -> s b h", h=HID)
                        if l == 1:
                            # u = dinv*relu(dinv*agg + b1)  [d, f] layout
                            b1exp = b1s[:].unsqueeze(1).broadcast_to(
                                [SLOTS, WB, HID]
                            )
                            ut = epool.tile([SLOTS, WB, HID], f32, tag="u1")
                            nc.vector.tensor_tensor(
                                out=ut[:], in0=accv, in1=dexp, op=Alu.mult,
                            )
                            nc.vector.tensor_tensor(
                                out=ut[:], in0=ut[:], in1=b1exp, op=Alu.add,
                            )
                            nc.vector.tensor_scalar(
                                out=ut[:], in0=ut[:], scalar1=0.0,
                                scalar2=None, op0=Alu.max,
                            )
                            uh = epool.tile([SLOTS, WB, HID], f32, tag="uh1")
                            nc.vector.tensor_tensor(
                                out=uh[:], in0=ut[:], in1=dexp, op=Alu.mult,
                            )
                            nc.sync.dma_start(
                                h1loc[w0 * SLOTS:(w0 + WB) * SLOTS, :]
                                .rearrange("(b s) h -> s b h", s=SLOTS),
                                uh[:],
                            )
                        else:
                            # acc holds agg^T [f, d] per window; W2 next
                            aggsb = epool.tile([HID, WB * SLOTS], f32,
                                               tag="aggsb")
                            nc.scalar.activation(aggsb[:], acc[:], Act.Copy)
                            bank2 = bpool.tile([SLOTS, WB * HID], f32,
                                               tag="accB")
                            for j in range(WB):
                                nc.tensor.matmul(
                                    out=bank2[:, j * HID:(j + 1) * HID],
                                    lhsT=aggsb[:,
                                               j * SLOTS:(j + 1) * SLOTS],
                                    rhs=w2s[:],
                                    start=True, stop=True,
                                )
                            b2exp = b2s[:].unsqueeze(1).broadcast_to(
                                [SLOTS, WB, HID]
                            )
                            ut = epool.tile([SLOTS, WB, HID], f32, tag="u2")
                            nc.vector.tensor_tensor(
                                out=ut[:],
                                in0=bank2[:].rearrange("s (b h) -> s b h",
                                                       h=HID),
                                in1=dexp, op=Alu.mult,
                            )
                            nc.vector.tensor_tensor(
                                out=ut[:], in0=ut[:], in1=b2exp, op=Alu.add,
                            )
                            nc.vector.tensor_scalar(
                                out=ut[:], in0=ut[:], scalar1=0.0,
                                scalar2=None, op0=Alu.max,
                            )
                            giexp = (
                                gids[:, w0:w0 + WB]
                                .unsqueeze(2).broadcast_to([SLOTS, WB, SG])
                            )
                            ioexp = iota52[:].unsqueeze(1).broadcast_to(
                                [SLOTS, WB, SG]
                            )
                            sgw = epool.tile([SLOTS, WB, SG], f32, tag="sgw")
                            nc.vector.tensor_tensor(
                                out=sgw[:], in0=ioexp, in1=giexp,
                                op=Alu.is_equal,
                            )
                            utf = ut[:].rearrange("s b h -> s (b h)")
                            for j in range(WB):
                                w = w0 + j
                                nc.tensor.matmul(
                                    out=pool_ps[:],
                                    lhsT=utf[:, j * HID:(j + 1) * HID],
                                    rhs=sgw[:, j, :],
                                    start=(w == 0), stop=(w == NW - 1),
                                )
                    if l == 2:
                        pst = epool.tile([HID, SG], f32, tag="pst")
                        nc.vector.tensor_copy(pst[:], pool_ps[:])
                        nc.sync.dma_start(pool_in_d[:], pst[:])

            layer(1, h0, fence0)

            cc_h1 = nc.gpsimd.collective_compute(
                "AllGather", Alu.bypass, replica_groups=groups,
                ins=[h1loc[:].rearrange("a b -> (a b)")],
                outs=[h1glob[:].rearrange("a b -> (a b)")],
            )

            layer(2, h1glob, cc_h1)

            # ---- pooled partial sums -> all-reduce -> final linear
            nc.gpsimd.collective_compute(
                "AllReduce", Alu.add, replica_groups=groups,
                ins=[pool_in_d[:]], outs=[pool_out_d[:]],
            )
            with (
                tc.tile_pool(name="fin", bufs=1) as fpool,
                tc.tile_pool(name="finps", bufs=1, space="PSUM") as fpsum,
            ):
                pr = fpool.tile([HID, SG], f32, tag="pr")
                nc.sync.dma_start(pr[:], pool_out_d[:])
                psc = fpool.tile([HID, SG], f32, tag="psc")
                nc.vector.tensor_tensor(
                    out=psc[:], in0=pr[:], in1=rcntb[0:HID, :], op=Alu.mult,
                )
                pso = fpsum.tile([SG, OUT_F], f32, tag="pso")
                nc.tensor.matmul(
                    out=pso[:], lhsT=psc[:], rhs=wos[:],
                    start=True, stop=True,
                )
                osb = fpool.tile([SG, OUT_F], f32, tag="osb")
                nc.vector.tensor_tensor(
                    out=osb[:], in0=pso[:], in1=bos[:], op=Alu.add,
                )
                nc.sync.dma_start(out[:], osb[0:N_GRAPHS, :])

    nc.compile()
    return nc


def kernel(x, edge_index, edge_attr, batch, W1, b1, W2, b2, Wo, bo, **_):
    per_core, plan, xt_virt, rcnt = _pack_host(x, edge_index, edge_attr, batch)
    nc = _build_program(plan)

    bo52 = np.zeros((SG, OUT_F), np.float32)
    bo52[:N_GRAPHS] = np.asarray(bo, np.float32).reshape(1, -1)
    common = dict(
        xt=xt_virt,
        w1=np.asarray(W1, np.float32),
        w2=np.asarray(W2, np.float32),
        wo=np.asarray(Wo, np.float32),
        b1bc=np.tile(np.asarray(b1, np.float32).reshape(1, -1), (SLOTS, 1)),
        b2bc=np.tile(np.asarray(b2, np.float32).reshape(1, -1), (SLOTS, 1)),
        bo52=bo52,
        rcnt=np.tile(rcnt.reshape(1, -1), (SLOTS, 1)),
    )
    in_maps = []
    for c in range(N_CORES):
        m = dict(common)
        m.update(per_core[c])
        in_maps.append(m)

    from concourse.bass_utils import run_bass_kernel_spmd

    res = run_bass_kernel_spmd(nc, in_maps, list(range(N_CORES)))
    out = res.results[0]["out"]
    kernel.last_exec_time_ns = res.exec_time_ns
    kernel.last_results = res.results
    kernel.last_res = res
    return np.asarray(out, np.float32)


kernel.last_exec_time_ns = None


# revision 4
# speedup vs baseline: 1.4143x; 1.1392x over previous
"""Trainium2 Bass kernel v3 for the 2-layer GCN (FCGraphGNN) over 8 NeuronCores.

v1 architecture (edge-sharded by dst, 64-node windows, dma_gather of 256B
message rows from per-core DRAM tables, one-hot S scatter matmuls, PSUM
segment-sum), with the measured bottlenecks fixed:
  - S one-hot tiles are fully host-precomputed (f32) and streamed from DRAM
    instead of 7k per-tile DVE builds (~4ms of DVE instruction overhead).
  - Epilogues, preamble scaling, and pooling are batched 8 windows wide
    (3 wide DVE ops per 8 windows instead of ~5 per window).
  - Graph-mean-pool counts are host-precomputed index data.
GPSIMD descriptor generation for the gathers (~7.3ns/edge) remains the
critical path; everything else is hidden underneath it.
"""

import os
import sys

import numpy as np

sys.path.insert(0, "/opt/trn_rl_repo")

N_NODES = 50000
N_EDGES = 3200000
N_GRAPHS = 50
IN_F = 5
HID = 64
OUT_F = 2
N_CORES = 8

SLOTS = 64          # dst nodes per window
T_SIDE = 17         # 128-edge tiles per (window, src-half)
CAP = T_SIDE * 128  # edge slots per (window, side)
NTW = 2 * T_SIDE    # tiles per window
GROUP_W = 2         # windows fetched per dma_gather pair
WB = 8              # windows per epilogue batch
SG = 52             # graph one-hot width (50 graphs + 2 junk bins)


def _pack_host(x, edge_index, edge_attr, batch):
    src = np.asarray(edge_index[0], dtype=np.int64)
    dst = np.asarray(edge_index[1], dtype=np.int64)
    ew = np.asarray(edge_attr, dtype=np.float32).reshape(-1)
    loop = np.arange(N_NODES, dtype=np.int64)
    src = np.concatenate([src, loop]).astype(np.int32)
    dst = np.concatenate([dst, loop]).astype(np.int32)
    ew = np.concatenate([ew, np.ones(N_NODES, np.float32)])
    E = src.shape[0]

    deg_cnt = np.bincount(dst, minlength=N_NODES).astype(np.int64)
    node_ptr = np.zeros(N_NODES + 1, np.int64)
    np.cumsum(deg_cnt, out=node_ptr[1:])
    order = np.argsort(dst, kind="stable")

    # core node boundaries balancing edge counts
    cum = node_ptr[1:]
    nb = [0]
    for c in range(1, N_CORES):
        nb.append(int(np.searchsorted(cum, c * E / N_CORES)))
    nb.append(N_NODES)
    nb = np.array(nb, np.int64)
    split_node = int(nb[4])  # src < split_node -> "lo" half of virtual space

    side_lo = src < split_node
    deg_lo = np.bincount(dst[side_lo], minlength=N_NODES).astype(np.int64)
    deg_hi = deg_cnt - deg_lo

    # window packing per core
    core_windows = []
    for c in range(N_CORES):
        wlist = []
        v = int(nb[c])
        end = int(nb[c + 1])
        while v < end:
            ws = v
            lo = hi = cnt = 0
            while (
                v < end
                and cnt < SLOTS
                and lo + deg_lo[v] <= CAP
                and hi + deg_hi[v] <= CAP
            ):
                lo += int(deg_lo[v])
                hi += int(deg_hi[v])
                cnt += 1
                v += 1
            wlist.append((ws, v))
        core_windows.append(wlist)

    NW = max(len(w) for w in core_windows)
    NW = (NW + WB - 1) // WB * WB
    assert NW <= 128, f"NW={NW} exceeds int16 index budget"
    NVC = NW * SLOTS
    NV = N_CORES * NVC
    NVH = NV // 2

    # vid map (node -> virtual id)
    node_vid = np.zeros(N_NODES, np.int32)
    for c in range(N_CORES):
        for w, (ws, we) in enumerate(core_windows[c]):
            node_vid[ws:we] = c * NVC + w * SLOTS + np.arange(we - ws,
                                                             dtype=np.int32)

    DEG_K = int(deg_cnt.max())
    DEG_K = (DEG_K + 3) // 4 * 4

    # per-side dst-sorted edge lists + ptrs
    lo_edges = order[side_lo[order]]
    hi_edges = order[~side_lo[order]]
    lo_ptr = np.zeros(N_NODES + 1, np.int64)
    np.cumsum(deg_lo, out=lo_ptr[1:])
    hi_ptr = np.zeros(N_NODES + 1, np.int64)
    np.cumsum(deg_hi, out=hi_ptr[1:])

    NTILES = NW * NTW
    NG = NW // GROUP_W
    GI = GROUP_W * CAP          # idxs per gather call
    IDXC = GI // 16

    vid_src = node_vid[src]

    # global position of each edge within its dst node's sorted run
    col_within = np.empty(E, np.int64)
    ar = np.arange(E, dtype=np.int64)
    col_within[order] = ar - node_ptr[dst[order]]

    # xt in virtual layout (shared by all cores)
    xt_virt = np.zeros((IN_F, NV), np.float32)
    xt_virt[:, node_vid] = np.asarray(x, np.float32).T

    batch_i = np.asarray(batch, np.int64)

    per_core = []
    for c in range(N_CORES):
        wlist = core_windows[c]
        ewcols = np.zeros((128, NTILES), np.float32)
        dstcols = np.zeros((128, NTILES), np.float32)
        idx_lo = np.zeros((NW, CAP), np.int16)
        idx_hi = np.zeros((NW, CAP), np.int16)
        gid = np.full((SLOTS, NW), 50.0, np.float32)
        ewdeg = np.zeros((NVC, DEG_K), np.float32)

        for w, (ws, we) in enumerate(wlist):
            for s, (edges, ptr, idxbuf, voff) in enumerate(
                ((lo_edges, lo_ptr, idx_lo, 0), (hi_edges, hi_ptr, idx_hi,
                                                 NVH))
            ):
                ids = edges[ptr[ws]:ptr[we]]
                n = ids.shape[0]
                t0 = (w * 2 + s) * T_SIDE
                j = np.arange(n)
                rows = j % 128
                cols = t0 + j // 128
                ewcols[rows, cols] = ew[ids]
                dstcols[rows, cols] = (dst[ids] - ws).astype(np.float32)
                idxbuf[w, :n] = (vid_src[ids] - voff).astype(np.int16)
            gid[: we - ws, w] = batch_i[ws:we].astype(np.float32)

        # padded per-node edge weights for the degree pass
        e_lo = int(node_ptr[nb[c]])
        e_hi = int(node_ptr[nb[c + 1]])
        es = order[e_lo:e_hi]
        rows = node_vid[dst[es]] - c * NVC
        ewdeg[rows, col_within[es]] = ew[es]
        rowdeg = np.zeros(NVC, np.int64)
        nr = node_vid[nb[c]:nb[c + 1]] - c * NVC
        rowdeg[nr] = deg_cnt[nb[c]:nb[c + 1]]
        ewdeg[rowdeg == 0, 0] = 1.0

        # wrap gather indices: [NG, 128, IDXC] (16-partition wrap, replicated)
        def wrap(a):
            g = a.reshape(NG, GI // 16, 16).transpose(0, 2, 1)  # [NG,16,IDXC]
            return np.ascontiguousarray(np.tile(g, (1, 8, 1)))

        per_core.append(
            dict(
                ewcols=ewcols,
                dstcols=dstcols,
                idxlo=wrap(idx_lo.reshape(-1)),
                idxhi=wrap(idx_hi.reshape(-1)),
                gid=np.ascontiguousarray(gid),
                ewdeg=ewdeg,
            )
        )

    cnt = np.bincount(batch_i, minlength=N_GRAPHS).astype(np.float32)
    rcnt = np.zeros(SG, np.float32)
    rcnt[:N_GRAPHS] = 1.0 / np.maximum(cnt, 1.0)

    plan = dict(
        NW=NW, NVC=NVC, NV=NV, NVH=NVH, DEG_K=DEG_K,
        NTILES=NTILES, NG=NG, GI=GI, IDXC=IDXC,
    )
    return per_core, plan, xt_virt, rcnt


def _build_program(plan):
    import concourse.bacc as bacc
    import concourse.tile as tile
    from concourse import mybir
    from concourse.tile_rust import add_dep_helper

    f32 = mybir.dt.float32
    i16 = mybir.dt.int16
    Alu = mybir.AluOpType
    Act = mybir.ActivationFunctionType

    NW = plan["NW"]; NVC = plan["NVC"]; NV = plan["NV"]; NVH = plan["NVH"]
    DEG_K = plan["DEG_K"]; NTILES = plan["NTILES"]; NG = plan["NG"]
    GI = plan["GI"]; IDXC = plan["IDXC"]

    NQ = int(os.environ.get("KNQ", "4"))
    nc = bacc.Bacc("TRN2", target_bir_lowering=False, debug=False,
                   num_devices=N_CORES, num_swdge_queues=NQ)

    xt = nc.declare_dram_parameter("xt", [IN_F, NV], f32, isOutput=False)
    w1 = nc.declare_dram_parameter("w1", [IN_F, HID], f32, isOutput=False)
    w2 = nc.declare_dram_parameter("w2", [HID, HID], f32, isOutput=False)
    wo = nc.declare_dram_parameter("wo", [HID, OUT_F], f32, isOutput=False)
    b1bc = nc.declare_dram_parameter("b1bc", [SLOTS, HID], f32, isOutput=False)
    b2bc = nc.declare_dram_parameter("b2bc", [SLOTS, HID], f32, isOutput=False)
    bo52 = nc.declare_dram_parameter("bo52", [SG, OUT_F], f32, isOutput=False)
    rcntp = nc.declare_dram_parameter("rcnt", [SLOTS, SG], f32, isOutput=False)
    ewdeg = nc.declare_dram_parameter("ewdeg", [NVC, DEG_K], f32,
                                      isOutput=False)
    ewcolsp = nc.declare_dram_parameter("ewcols", [128, NTILES], f32,
                                        isOutput=False)
    dstcolsp = nc.declare_dram_parameter("dstcols", [128, NTILES], f32,
                                         isOutput=False)
    idxlo = nc.declare_dram_parameter("idxlo", [NG, 128, IDXC], i16,
                                      isOutput=False)
    idxhi = nc.declare_dram_parameter("idxhi", [NG, 128, IDXC], i16,
                                      isOutput=False)
    gidp = nc.declare_dram_parameter("gid", [SLOTS, NW], f32, isOutput=False)
    out = nc.declare_dram_parameter("out", [N_GRAPHS, OUT_F], f32,
                                    isOutput=True)

    groups = [list(range(N_CORES))]

    with tile.TileContext(nc) as tc:
        with (
            tc.tile_pool(name="dram", bufs=1, space="DRAM") as dram,
            tc.tile_pool(name="const", bufs=1) as cpool,
            tc.tile_pool(name="persist", bufs=1) as ppool,
        ):
            h0 = dram.tile([NV, HID], f32, tag="h0")
            h1loc = dram.tile([NVC, HID], f32, tag="h1loc")
            h1glob = dram.tile([NV, HID], f32, tag="h1glob")
            dinv_loc_d = dram.tile([SLOTS, NW], f32, tag="dinvloc")
            dinv_glob_d = dram.tile([N_CORES, SLOTS, NW], f32, tag="dinvglob")
            pool_in_d = dram.tile([HID, SG], f32, tag="poolin")
            pool_out_d = dram.tile([HID, SG], f32, tag="poolout")

            # ---- constants
            iota52 = cpool.tile([SLOTS, SG], f32, tag="iota52")
            nc.gpsimd.iota(iota52[:], pattern=[[1, SG]], base=0,
                           channel_multiplier=0,
                           allow_small_or_imprecise_dtypes=True)
            iota64 = cpool.tile([128, SLOTS], f32, tag="iota64")
            nc.gpsimd.iota(iota64[:], pattern=[[1, SLOTS]], base=0,
                           channel_multiplier=0,
                           allow_small_or_imprecise_dtypes=True)
            ewc = cpool.tile([128, NTILES], f32, tag="ewc")
            nc.sync.dma_start(ewc[:], ewcolsp[:])
            dstc = cpool.tile([128, NTILES], f32, tag="dstc")
            nc.sync.dma_start(dstc[:], dstcolsp[:])
            w1s = cpool.tile([IN_F, HID], f32, tag="w1s")
            nc.sync.dma_start(w1s[:], w1[:])
            w2s = cpool.tile([HID, HID], f32, tag="w2s")
            nc.sync.dma_start(w2s[:], w2[:])
            wos = cpool.tile([HID, OUT_F], f32, tag="wos")
            nc.sync.dma_start(wos[:], wo[:])
            b1s = cpool.tile([SLOTS, HID], f32, tag="b1s")
            nc.sync.dma_start(b1s[:], b1bc[:])
            b2s = cpool.tile([SLOTS, HID], f32, tag="b2s")
            nc.sync.dma_start(b2s[:], b2bc[:])
            bos = cpool.tile([SG, OUT_F], f32, tag="bos")
            nc.sync.dma_start(bos[:], bo52[:])
            rcntb = cpool.tile([SLOTS, SG], f32, tag="rcntb")
            nc.sync.dma_start(rcntb[:], rcntp[:])
            gids = cpool.tile([SLOTS, NW], f32, tag="gids")
            nc.sync.dma_start(gids[:], gidp[:])

            dinvw = ppool.tile([SLOTS, NW], f32, tag="dinvw")
            dinvg = ppool.tile([SLOTS, N_CORES, NW], f32, tag="dinvg")

            # ---- degree pass -> local dinv
            with tc.tile_pool(name="deg", bufs=1) as dpool:
                degt = dpool.tile([SLOTS, NW, DEG_K], f32, tag="degt")
                nc.sync.dma_start(
                    degt[:], ewdeg[:].rearrange("(w s) k -> s w k", s=SLOTS)
                )
                deg = dpool.tile([SLOTS, NW], f32, tag="deg")
                nc.vector.tensor_reduce(
                    out=deg[:], in_=degt[:], axis=mybir.AxisListType.X,
                    op=Alu.add,
                )
                rec = dpool.tile([SLOTS, NW], f32, tag="rec")
                nc.vector.reciprocal(rec[:], deg[:])
                nc.scalar.activation(dinvw[:], rec[:], Act.Sqrt)
                nc.sync.dma_start(dinv_loc_d[:], dinvw[:])

            # all-gather dinv
            nc.gpsimd.collective_compute(
                "AllGather", Alu.bypass, replica_groups=groups,
                ins=[dinv_loc_d[:].rearrange("s w -> (s w)")],
                outs=[dinv_glob_d[:].rearrange("r s w -> (r s w)")],
            )
            nc.sync.dma_start(
                dinvg[:], dinv_glob_d[:].rearrange("r s w -> s r w")
            )

            # ---- preamble: h0 = dinv * (x @ W1), all rows, per core
            with (
                tc.tile_pool(name="pre", bufs=2) as prepool,
                tc.tile_pool(name="prepsum", bufs=2, space="PSUM") as pspool,
                tc.tile_pool(name="prestage", bufs=2) as stpool,
            ):
                BW = 8  # window tiles per h0 write
                h0_writes = {0: [], 1: []}
                for r in (0, 1, 2, 3, 4, 5, 6, 7):
                    xts = prepool.tile([IN_F, NVC], f32, tag="xts")
                    nc.sync.dma_start(
                        xts[:], xt[:, r * NVC:(r + 1) * NVC]
                    )
                    for wb in range(NW // BW):
                        bank = pspool.tile([SLOTS, BW * HID], f32,
                                           tag="prebank")
                        for j in range(BW):
                            w = wb * BW + j
                            nc.tensor.matmul(
                                out=bank[:, j * HID:(j + 1) * HID],
                                lhsT=xts[:, w * SLOTS:(w + 1) * SLOTS],
                                rhs=w1s[:],
                                start=True, stop=True,
                            )
                        stage = stpool.tile([SLOTS, BW, HID], f32, tag="h0st")
                        dexp = (
                            dinvg[:, r, wb * BW:(wb + 1) * BW]
                            .unsqueeze(2).broadcast_to([SLOTS, BW, HID])
                        )
                        nc.vector.tensor_tensor(
                            out=stage[:],
                            in0=bank[:].rearrange("s (b h) -> s b h", h=HID),
                            in1=dexp, op=Alu.mult,
                        )
                        row0 = (r * NW + wb * BW) * SLOTS
                        h0_writes[r // 4].append(
                            nc.sync.dma_start(
                                h0[row0:row0 + BW * SLOTS, :].rearrange(
                                    "(b s) h -> s b h", s=SLOTS
                                ),
                                stage[:],
                            )
                        )

            # fences: lo/hi gathers wait only on their half of h0
            fence0_lo = nc.gpsimd.engine_nop()
            for wi in h0_writes[0]:
                add_dep_helper(fence0_lo.ins, wi.ins,
                               reason="h0 lo half ready")
            fence0_hi = nc.gpsimd.engine_nop()
            for wi in h0_writes[1]:
                add_dep_helper(fence0_hi.ins, wi.ins,
                               reason="h0 hi half ready")

            # ---- message-passing layer (one pass over the edge tiles)
            def layer(l, src_table, fence_lo, fence_hi):
                lo_view = src_table[0:NVH, :]
                hi_view = src_table[NVH:NV, :]
                with (
                    tc.tile_pool(name=f"idx{l}", bufs=4) as ipool,
                    tc.tile_pool(name=f"mbuf{l}", bufs=3) as mpool,
                    tc.tile_pool(name=f"sstr{l}", bufs=3) as spool,
                    tc.tile_pool(name=f"acc{l}", bufs=2, space="PSUM") as apool,
                    tc.tile_pool(name=f"acc2{l}", bufs=2, space="PSUM") as bpool,
                    tc.tile_pool(name=f"epi{l}", bufs=2) as epool,
                    tc.tile_pool(name=f"gp{l}", bufs=1, space="PSUM") as gpool,
                ):
                    if l == 2:
                        pool_ps = gpool.tile([HID, SG], f32, tag="poolps")
                    for wbi in range(NW // WB):
                        acc = apool.tile([SLOTS, WB * HID], f32, tag="accA")
                        for gg in range(WB // GROUP_W):
                            g = wbi * (WB // GROUP_W) + gg
                            ilo = ipool.tile([128, IDXC], i16, tag="ilo")
                            nc.sync.dma_start(ilo[:], idxlo[g])
                            ihi = ipool.tile([128, IDXC], i16, tag="ihi")
                            nc.sync.dma_start(ihi[:], idxhi[g])
                            mlo = mpool.tile(
                                [128, GROUP_W * T_SIDE, HID], f32, tag="mlo"
                            )
                            mhi = mpool.tile(
                                [128, GROUP_W * T_SIDE, HID], f32, tag="mhi"
                            )
                            glo = nc.gpsimd.dma_gather(
                                mlo[:], lo_view, ilo[:], GI, GI, HID,
                                single_packet=False,
                                queue_num=(2 * g) % NQ,
                            )
                            ghi = nc.gpsimd.dma_gather(
                                mhi[:], hi_view, ihi[:], GI, GI, HID,
                                single_packet=False,
                                queue_num=(2 * g + 1) % NQ,
                            )
                            if fence_lo is not None:
                                add_dep_helper(glo.ins, fence_lo.ins,
                                               reason="src table ready")
                                add_dep_helper(ghi.ins, fence_hi.ins,
                                               reason="src table ready")
                            ssb = spool.tile(
                                [128, GROUP_W * NTW, SLOTS], f32, tag="ssb"
                            )
                            t0 = g * GROUP_W * NTW
                            nt = GROUP_W * NTW
                            ioexp64 = iota64[:].unsqueeze(1).broadcast_to(
                                [128, nt, SLOTS]
                            )
                            dexpc = dstc[:, t0:t0 + nt].unsqueeze(2) \
                                .broadcast_to([128, nt, SLOTS])
                            eexpc = ewc[:, t0:t0 + nt].unsqueeze(2) \
                                .broadcast_to([128, nt, SLOTS])
                            nc.vector.tensor_tensor(
                                out=ssb[:], in0=ioexp64, in1=dexpc,
                                op=Alu.is_equal,
                            )
                            nc.vector.tensor_tensor(
                                out=ssb[:], in0=ssb[:], in1=eexpc,
                                op=Alu.mult,
                            )
                            for wl in range(GROUP_W):
                                wj = gg * GROUP_W + wl  # window in batch
                                aslice = acc[:, wj * HID:(wj + 1) * HID]
                                k = 0
                                for s, mb in ((0, mlo), (1, mhi)):
                                    for ti in range(T_SIDE):
                                        blk = wl * T_SIDE + ti
                                        st = wl * NTW + s * T_SIDE + ti
                                        first = k == 0
                                        last = k == NTW - 1
                                        if l == 1:
                                            nc.tensor.matmul(
                                                out=aslice,
                                                lhsT=ssb[:, st, :],
                                                rhs=mb[:, blk, :],
                                                start=first, stop=last,
                                            )
                                        else:
                                            nc.tensor.matmul(
                                                out=aslice,
                                                lhsT=mb[:, blk, :],
                                                rhs=ssb[:, st, :],
                                                start=first, stop=last,
                                            )
                                        k += 1
                        # ---- batched epilogue over WB windows
                        w0 = wbi * WB
                        dexp = (
                            dinvw[:, w0:w0 + WB]
                            .unsqueeze(2).broadcast_to([SLOTS, WB, HID])
                        )
                        accv = acc[:].rearrange("s (b h) -> s b h", h=HID)
                        if l == 1:
                            # u = dinv*relu(dinv*agg + b1)  [d, f] layout
                            b1exp = b1s[:].unsqueeze(1).broadcast_to(
                                [SLOTS, WB, HID]
                            )
                            ut = epool.tile([SLOTS, WB, HID], f32, tag="u1")
                            nc.vector.tensor_tensor(
                                out=ut[:], in0=accv, in1=dexp, op=Alu.mult,
                            )
                            nc.vector.tensor_tensor(
                                out=ut[:], in0=ut[:], in1=b1exp, op=Alu.add,
                            )
                            nc.vector.tensor_scalar(
                                out=ut[:], in0=ut[:], scalar1=0.0,
                                scalar2=None, op0=Alu.max,
                            )
                            uh = epool.tile([SLOTS, WB, HID], f32, tag="uh1")
                            nc.vector.tensor_tensor(
                                out=uh[:], in0=ut[:], in1=dexp, op=Alu.mult,
                            )
                            nc.sync.dma_start(
                                h1loc[w0 * SLOTS:(w0 + WB) * SLOTS, :]
                                .rearrange("(b s) h -> s b h", s=SLOTS),
                                uh[:],
                            )
                        else:
                            # acc holds agg^T [f, d] per window; W2 next
                            aggsb = epool.tile([HID, WB * SLOTS], f32,
                                               tag="aggsb")
                            nc.scalar.activation(aggsb[:], acc[:], Act.Copy)
                            bank2 = bpool.tile([SLOTS, WB * HID], f32,
                                               tag="accB")
                            for j in range(WB):
                                nc.tensor.matmul(
                                    out=bank2[:, j * HID:(j + 1) * HID],
                                    lhsT=aggsb[:,
                                               j * SLOTS:(j + 1) * SLOTS],
                                    rhs=w2s[:],
                                    start=True, stop=True,
                                )
                            b2exp = b2s[:].unsqueeze(1).broadcast_to(
                                [SLOTS, WB, HID]
                            )
                            ut = epool.tile([SLOTS, WB, HID], f32, tag="u2")
                            nc.vector.tensor_tensor(
                                out=ut[:],
                                in0=bank2[:].rearrange("s (b h) -> s b h",
                                                       h=HID),
                                in1=dexp, op=Alu.mult,
                            )
                            nc.vector.tensor_tensor(
                                out=ut[:], in0=ut[:], in1=b2exp, op=Alu.add,
                            )
                            nc.vector.tensor_scalar(
                                out=ut[:], in0=ut[:], scalar1=0.0,
                                scalar2=None, op0=Alu.max,
                            )
                            giexp = (
                                gids[:, w0:w0 + WB]
                                .unsqueeze(2).broadcast_to([SLOTS, WB, SG])
                            )
                            ioexp = iota52[:].unsqueeze(1).broadcast_to(
                                [SLOTS, WB, SG]
                            )
                            sgw = epool.tile([SLOTS, WB, SG], f32, tag="sgw")
                            nc.vector.tensor_tensor(
                                out=sgw[:], in0=ioexp, in1=giexp,
                                op=Alu.is_equal,
                            )
                            utf = ut[:].rearrange("s b h -> s (b h)")
                            for j in range(WB):
                                w = w0 + j
                                nc.tensor.matmul(
                                    out=pool_ps[:],
                                    lhsT=utf[:, j * HID:(j + 1) * HID],
                                    rhs=sgw[:, j, :],
                                    start=(w == 0), stop=(w == NW - 1),
                                )
                    if l == 2:
                        pst = epool.tile([HID, SG], f32, tag="pst")
                        nc.vector.tensor_copy(pst[:], pool_ps[:])
                        nc.sync.dma_start(pool_in_d[:], pst[:])

            layer(1, h0, fence0_lo, fence0_hi)

            cc_h1 = nc.gpsimd.collective_compute(
                "AllGather", Alu.bypass, replica_groups=groups,
                ins=[h1loc[:].rearrange("a b -> (a b)")],
                outs=[h1glob[:].rearrange("a b -> (a b)")],
            )

            layer(2, h1glob, cc_h1, cc_h1)

            # ---- pooled partial sums -> all-reduce -> final linear
            nc.gpsimd.collective_compute(
                "AllReduce", Alu.add, replica_groups=groups,
                ins=[pool_in_d[:]], outs=[pool_out_d[:]],
            )
            with (
                tc.tile_pool(name="fin", bufs=1) as fpool,
                tc.tile_pool(name="finps", bufs=1, space="PSUM") as fpsum,
            ):
                pr = fpool.tile([HID, SG], f32, tag="pr")
                nc.sync.dma_start(pr[:], pool_out_d[:])
                psc = fpool.tile([HID, SG], f32, tag="psc")
                nc.vector.tensor_tensor(
                    out=psc[:], in0=pr[:], in1=rcntb[0:HID, :], op=Alu.mult,
                )
                pso = fpsum.tile([SG, OUT_F], f32, tag="pso")
                nc.tensor.matmul(
                    out=pso[:], lhsT=psc[:], rhs=wos[:],
                    start=True, stop=True,
                )
                osb = fpool.tile([SG, OUT_F], f32, tag="osb")
                nc.vector.tensor_tensor(
                    out=osb[:], in0=pso[:], in1=bos[:], op=Alu.add,
                )
                nc.sync.dma_start(out[:], osb[0:N_GRAPHS, :])

    nc.compile()
    return nc


def kernel(x, edge_index, edge_attr, batch, W1, b1, W2, b2, Wo, bo, **_):
    per_core, plan, xt_virt, rcnt = _pack_host(x, edge_index, edge_attr, batch)
    nc = _build_program(plan)

    bo52 = np.zeros((SG, OUT_F), np.float32)
    bo52[:N_GRAPHS] = np.asarray(bo, np.float32).reshape(1, -1)
    common = dict(
        xt=xt_virt,
        w1=np.asarray(W1, np.float32),
        w2=np.asarray(W2, np.float32),
        wo=np.asarray(Wo, np.float32),
        b1bc=np.tile(np.asarray(b1, np.float32).reshape(1, -1), (SLOTS, 1)),
        b2bc=np.tile(np.asarray(b2, np.float32).reshape(1, -1), (SLOTS, 1)),
        bo52=bo52,
        rcnt=np.tile(rcnt.reshape(1, -1), (SLOTS, 1)),
    )
    in_maps = []
    for c in range(N_CORES):
        m = dict(common)
        m.update(per_core[c])
        in_maps.append(m)

    from concourse.bass_utils import run_bass_kernel_spmd

    res = run_bass_kernel_spmd(nc, in_maps, list(range(N_CORES)))
    out = res.results[0]["out"]
    kernel.last_exec_time_ns = res.exec_time_ns
    kernel.last_results = res.results
    kernel.last_res = res
    return np.asarray(out, np.float32)


kernel.last_exec_time_ns = None
